# revision 12
# baseline (speedup 1.0000x reference)
"""Multi-head attention (dense_transformer) on 8 TRN2 NeuronCores.

Reference computation (B=1, N=4096, D=512, 8 heads, head_dim 64):
    q = x @ Wq.T ; k, v = split(x @ Wkv.T)
    attn = softmax_masked(q k^T * scale)   # diagonal masked to zero
    out = (attn @ v) @ Wproj.T + bproj
Sharding: head-parallel - core h computes head h end to end including its
partial output projection; the host sums the 8 partials and adds the bias.

Per-core layout is fully "transposed" (channels on partitions):
    XT  [c=512, n=4096]    via 20 xbar DMA transposes
    QT/KT [128, 4096]      rows 0..63 = head projection, rows 64..127 = copy
                           (duplicated halves enable 2-way row-packed QK)
    ST group g = scores for key strips 2g, 2g+1 -> [128 keys, 2, 512 q] PSUM
    E = exp(scale * ST): split between ScalarE (exact table exp) and DVE
        (Schraudolph bitcast exp: bf16bits = round(A*s + B), one fused
        multiply-add at full DVE rate) so the two engines share the
        16.7M-element exp stream.  GpSimd zeroes the diagonal blocks.
    PV split-K: keys 0..63 and 64..127 of each strip feed two CONCURRENT
        row-tiled matmuls (PE quadrants (0,0) and (64,0)) accumulating
        OTa/OTb [65, 512] in separate PSUM banks; row 64 = sum of exps.
    merge: ScalarE evacuates OTb (+OTa's sums row), DVE adds PSUM+SBUF,
        GpSimd merges the sums rows.
    PO[tok, 512] = OTS[0:64, tok-block].T @ WprojSlice^T
    out[tok, :]  = PO * (1/sums)[tok]  (alternating DVE / ScalarE)
"""

import numpy as np

import concourse.bass as bass
import concourse.tile as tile
from concourse import bacc, mybir
from concourse.bass_utils import run_bass_kernel_spmd

F32 = mybir.dt.float32
BF16 = mybir.dt.bfloat16
I16 = mybir.dt.int16
EXP = mybir.ActivationFunctionType.Exp
COPY = mybir.ActivationFunctionType.Copy
MULT = mybir.AluOpType.mult
ADD = mybir.AluOpType.add

N = 4096
D = 512
NH = 8
HD = 64
NQC = 8          # query chunks of 512
QC = 512
NST = 32         # key strips of 128
GS = 2           # key strips per exp group
NG = NST // GS   # 16 groups per chunk

# Schraudolph bf16-bitcast exp: bf16_bits(exp(x)) ~ round(A16*x + B16)
A16 = 128.0 / float(np.log(2.0))
B16 = 16248.76   # mean-centered (HW-calibrated; RNE convert)

LAST_EXEC_TIME_NS = None
_BUILD_CACHE = {}


def _dve_exp_group(c, g):
    """Which exp groups run on the DVE (Schraudolph) vs ScalarE."""
    if c == 0:
        # chunk 0: DVE is busy evacuating the qt/kt/vp preamble
        return g % 4 == 3
    return g % 2 == 1


def _build(scale_val: float):
    nc = bacc.Bacc("TRN2", target_bir_lowering=False, debug=False)

    x_d = nc.dram_tensor("xinp", [N, D], BF16, kind="ExternalInput").ap()
    wq_d = nc.dram_tensor("wq", [128, 4, 128], BF16, kind="ExternalInput").ap()
    wk_d = nc.dram_tensor("wk", [128, 4, 128], BF16, kind="ExternalInput").ap()
    wv_d = nc.dram_tensor("wv", [128, 4, HD + 2], BF16, kind="ExternalInput").ap()
    mask_d = nc.dram_tensor("mask", [128, 128], BF16, kind="ExternalInput").ap()
    wp_d = nc.dram_tensor("wp", [HD, D], BF16, kind="ExternalInput").ap()
    ident_d = nc.dram_tensor("ident", [128, 1], F32, kind="ExternalInput").ap()
    out_d = nc.dram_tensor("out", [N, D], BF16, kind="ExternalOutput").ap()

    a_s = A16 * scale_val

    with tile.TileContext(nc) as tc:
        with (
            tc.tile_pool(name="consts", bufs=1) as consts,
            tc.tile_pool(name="persist", bufs=1) as persist,
            tc.tile_pool(name="epool", bufs=6) as epool,
            tc.tile_pool(name="small", bufs=2) as small,
            tc.tile_pool(name="otbs", bufs=2) as otbs,
            tc.tile_pool(name="outp", bufs=4) as outp,
            tc.tile_pool(name="ps_st", bufs=2, space="PSUM") as ps_st,
            tc.tile_pool(name="ps_ot", bufs=1, space="PSUM") as ps_ot,
            tc.tile_pool(name="ps_misc", bufs=2, space="PSUM") as ps_misc,
        ):
            # ---- constants ----
            wq_sb = consts.tile([128, 4, 128], BF16, tag="wq")
            wk_sb = consts.tile([128, 4, 128], BF16, tag="wk")
            wv_sb = consts.tile([128, 4, HD + 2], BF16, tag="wv")
            mask_sb = consts.tile([128, 128], BF16, tag="mask")
            wp_sb = consts.tile([HD, D], BF16, tag="wp")
            identb1_sb = consts.tile([128, 1], F32, tag="ident")
            nc.sync.dma_start(out=wk_sb, in_=wk_d)
            nc.sync.dma_start(out=wq_sb, in_=wq_d)

            # ---- persistent tensors ----
            xt = persist.tile([128, 4, N], BF16, tag="xt")     # XT[c%128, c//128, n]
            qt = persist.tile([128, N], BF16, tag="qt")        # QT duplicated halves
            kt = persist.tile([128, N], BF16, tag="kt")
            vp = persist.tile([128, NST, HD + 2], BF16, tag="vp")

            # x arrives transposed via the xbar; the first chunk's tokens
            # come in 512-token tiles so chunk 0 starts early.
            for b0, bn in ((0, 512), (512, 512), (1024, 1024),
                           (2048, 1024), (3072, 1024)):
                for cb in range(4):
                    nc.sync.dma_start_transpose(
                        out=xt[:, cb, b0:b0 + bn],
                        in_=x_d[b0:b0 + bn, cb * 128:(cb + 1) * 128],
                    )

            nc.sync.dma_start(out=wv_sb, in_=wv_d)
            nc.sync.dma_start(out=mask_sb, in_=mask_d)
            nc.sync.dma_start(out=wp_sb, in_=wp_d)
            nc.sync.dma_start(out=identb1_sb, in_=ident_d)

            # preload the exp activation table while DMAs stream
            scratch = consts.tile([1, 2], F32, tag="scratch")
            nc.vector.memset(scratch, 0.0)
            nc.scalar.activation(scratch, scratch, EXP)
            # ones column of V' (row 64 of every strip) written once
            nc.vector.memset(vp[:, :, HD:HD + 1], 1.0)

            kt_done = [False] * NQC
            qt_done = [False] * NQC
            vq_done = [False] * (NST // 4)

            def prod_kt(qc):
                if kt_done[qc]:
                    return
                kt_done[qc] = True
                sl = slice(qc * QC, (qc + 1) * QC)
                pp = ps_misc.tile([128, QC], F32, tag="misc", name=f"ktp{qc}")
                for cc in range(4):
                    nc.tensor.matmul(
                        pp, wk_sb[:, cc, :], xt[:, cc, sl],
                        start=(cc == 0), stop=(cc == 3),
                    )
                nc.vector.tensor_copy(kt[:, sl], pp)

            def prod_qt(qc):
                if qt_done[qc]:
                    return
                qt_done[qc] = True
                sl = slice(qc * QC, (qc + 1) * QC)
                pp = ps_misc.tile([128, QC], F32, tag="misc", name=f"qtp{qc}")
                for cc in range(4):
                    nc.tensor.matmul(
                        pp, wq_sb[:, cc, :], xt[:, cc, sl],
                        start=(cc == 0), stop=(cc == 3),
                    )
                nc.vector.tensor_copy(qt[:, sl], pp)

            def prod_vq(q):
                """V' for strips 4q..4q+3 in one PSUM tile + one copy."""
                if vq_done[q]:
                    return
                vq_done[q] = True
                vv = ps_misc.tile([128, 4, HD + 2], F32, tag="misc",
                                  name=f"vv{q}")
                for t in range(4):
                    for cc in range(4):
                        nc.tensor.matmul(
                            vv[:, t, :],
                            xt[:, cc, (4 * q + t) * 128:(4 * q + t + 1) * 128],
                            wv_sb[:, cc, :],
                            start=(cc == 0), stop=(cc == 3),
                        )
                nc.vector.tensor_copy(
                    vp[:, 4 * q:4 * q + 4, 0:HD], vv[:, :, 0:HD])

            def prod_for_group(c, g):
                if c > 0 or g >= NG:
                    return
                prod_kt(g // 2)
                prod_vq(g // 2)

            # ---- per-chunk state ----
            ota_tiles = {}
            otb_tiles = {}
            e_tiles = {}
            st_tiles = {}
            sums_tiles = {}
            ots_tiles = {}

            def emit_qk(c, g):
                qsl = slice(c * QC, (c + 1) * QC)
                st = ps_st.tile([128, GS, QC], F32, tag="st")
                st_tiles[(c, g)] = st
                for i in range(GS):
                    j = GS * g + i
                    ro = 64 * (j % 2)
                    nc.tensor.matmul(
                        st[:, i, :],
                        kt[ro:ro + 64, j * 128:(j + 1) * 128],
                        qt[ro:ro + 64, qsl],
                        start=True,
                        stop=True,
                    )

            def emit_exp(c, g):
                e_t = epool.tile([128, GS, QC], BF16, tag="e")
                e_tiles[(c, g)] = e_t
                st = st_tiles.pop((c, g))
                if _dve_exp_group(c, g):
                    nc.vector.tensor_scalar(
                        e_t.bitcast(I16), st, a_s, B16, MULT, ADD)
                else:
                    nc.scalar.activation(e_t, st, EXP, scale=scale_val)
                for i in range(GS):
                    j = GS * g + i
                    r = j - 4 * c
                    if 0 <= r < 4:
                        blk = e_t[:, i, r * 128:(r + 1) * 128]
                        nc.gpsimd.tensor_tensor(blk, blk, mask_sb, MULT)

            def emit_pv(c, g):
                if g == 0:
                    ota_tiles[c] = ps_ot.tile([HD + 1, QC], F32, tag="ota",
                                              name=f"ota{c}")
                    otb_tiles[c] = ps_ot.tile([HD + 1, QC], F32, tag="otb",
                                              name=f"otb{c}")
                ota = ota_tiles[c]
                otb = otb_tiles[c]
                e_t = e_tiles.pop((c, g))
                for i in range(GS):
                    j = GS * g + i
                    nc.tensor.matmul(
                        ota,
                        vp[0:64, j, 0:HD + 1],
                        e_t[0:64, i, :],
                        start=(j == 0),
                        stop=(j == NST - 1),
                        skip_group_check=True,
                    )
                    nc.tensor.matmul(
                        otb,
                        vp[64:128, j, 0:HD + 1],
                        e_t[64:128, i, :],
                        start=(j == 0),
                        stop=(j == NST - 1),
                        skip_group_check=True,
                    )

            def emit_copies(c):
                # drain OTa/OTb out of PSUM; ScalarE evacuates the b half,
                # one DVE add merges rows 0..64 (row 64 = sums, bf16).
                ota = ota_tiles.pop(c)
                otb = otb_tiles.pop(c)
                otb_sb = otbs.tile([HD + 1, QC], F32, tag="otbs")
                nc.scalar.activation(otb_sb, otb, COPY)
                ots_sb = small.tile([HD, QC], BF16, tag="ots")
                nc.vector.tensor_tensor(
                    ots_sb, ota[0:HD, :], otb_sb[0:HD, :], ADD)
                sums_t = small.tile([HD + 1, QC], F32, tag="sums")
                nc.vector.tensor_tensor(
                    sums_t[HD:HD + 1, :], ota[HD:HD + 1, :],
                    otb_sb[HD:HD + 1, :], ADD)
                sums_tiles[c] = sums_t
                ots_tiles[c] = ots_sb

            def emit_norm_po(c):
                pool = ps_st if c == NQC - 1 else ps_misc
                ptag = "st" if c == NQC - 1 else "misc"
                ots_sb = ots_tiles.pop(c)
                sums_t = sums_tiles.pop(c)
                ts_ps = pool.tile([128, 4], F32, tag=ptag, name=f"ts{c}")
                for tb in range(4):
                    nc.tensor.transpose(
                        ts_ps[:, tb:tb + 1],
                        sums_t[HD:HD + 1, tb * 128:(tb + 1) * 128],
                        identb1_sb[HD:HD + 1, 0:1],
                    )
                recip_sb = small.tile([128, 4], F32, tag="recip")
                with nc.allow_low_precision(reason="fp32 reciprocal"):
                    nc.vector.reciprocal(recip_sb, ts_ps)
                for tb in range(4):
                    po = pool.tile([128, QC], F32, tag=ptag, name=f"po{c}_{tb}")
                    nc.tensor.matmul(
                        po,
                        ots_sb[:, tb * 128:(tb + 1) * 128],
                        wp_sb,
                        start=True,
                        stop=True,
                    )
                    o_sb = outp.tile([128, D], BF16, tag="outs")
                    if tb % 2 == 0:
                        nc.vector.tensor_scalar_mul(
                            o_sb, po, recip_sb[:, tb:tb + 1])
                    else:
                        nc.scalar.activation(
                            o_sb, po, COPY, scale=recip_sb[:, tb:tb + 1])
                    row = c * QC + tb * 128
                    nc.sync.dma_start(out=out_d[row:row + 128, :], in_=o_sb)

            # ---- flat software pipeline across all (chunk, group) steps ----
            seq = [(c, g) for c in range(NQC) for g in range(NG)]
            prod_qt(0)
            prod_for_group(0, 0)
            for i, (c, g) in enumerate(seq):
                emit_qk(c, g)
                if i > 1:
                    pc, pg = seq[i - 2]
                    emit_pv(pc, pg)
                    if pg == NG - 1:
                        emit_copies(pc)
                emit_exp(c, g)
                prod_for_group(c, g + 1)
                prod_for_group(c, g + 2)
                if i > 3:
                    ppc, ppg = seq[i - 4]
                    if ppg == NG - 1:
                        emit_norm_po(ppc)
                if g == NG // 2:
                    prod_qt(min(c + 1, NQC - 1))
            for i in (len(seq) - 2, len(seq) - 1):
                emit_pv(*seq[i])
            emit_copies(NQC - 1)
            emit_norm_po(NQC - 1)

    nc.compile()
    return nc


def _prep_inputs(x, scale, Wq, Wkv, Wproj):
    """Per-core input maps (head h on core h)."""
    import ml_dtypes
    bf = ml_dtypes.bfloat16
    x2 = np.ascontiguousarray(x.reshape(N, D)).astype(bf)
    mask = (1.0 - np.eye(128)).astype(np.float32)
    ident = np.ones((128, 1), dtype=np.float32)
    in_maps = []
    for h in range(NH):
        wqh = Wq[h * HD:(h + 1) * HD, :]                  # [64, 512]
        wkh = Wkv[h * HD:(h + 1) * HD, :]
        wvh = Wkv[D + h * HD:D + (h + 1) * HD, :]
        # lhsT [c, m] with m duplicated halves -> [128, 4x128]
        def lhsT_dup(w):
            a = np.concatenate([w.T, w.T], axis=1)        # [512, 128]
            return np.ascontiguousarray(
                a.reshape(4, 128, 128).transpose(1, 0, 2))
        # V' rhs [c, 66] -> [128, 4, 66] (col 64 becomes the ones column)
        b = np.concatenate(
            [wvh.T, np.zeros((D, 2), dtype=np.float32)], axis=1)
        wv_host = np.ascontiguousarray(
            b.reshape(4, 128, HD + 2).transpose(1, 0, 2))
        wp_host = np.ascontiguousarray(
            Wproj[:, h * HD:(h + 1) * HD].T, dtype=np.float32)  # [64, 512]
        in_maps.append({
            "xinp": x2,
            "wq": np.ascontiguousarray(lhsT_dup(wqh)).astype(bf),
            "wk": np.ascontiguousarray(lhsT_dup(wkh)).astype(bf),
            "wv": np.ascontiguousarray(wv_host).astype(bf),
            "mask": mask.astype(bf),
            "wp": wp_host.astype(bf),
            "ident": ident,
        })
    return in_maps


def kernel(x, H, W, scale, Wq, Wkv, Wproj, bproj, _trace=False):
    global LAST_EXEC_TIME_NS
    x = np.asarray(x, dtype=np.float32)
    Wq = np.asarray(Wq, dtype=np.float32)
    Wkv = np.asarray(Wkv, dtype=np.float32)
    Wproj = np.asarray(Wproj, dtype=np.float32)
    bproj = np.asarray(bproj, dtype=np.float32)
    scale_val = float(np.asarray(scale).reshape(-1)[0])

    key = round(scale_val, 12)
    nc = _BUILD_CACHE.get(key)
    if nc is None:
        nc = _build(scale_val)
        _BUILD_CACHE[key] = nc

    in_maps = _prep_inputs(x, scale, Wq, Wkv, Wproj)
    try:
        res = run_bass_kernel_spmd(
            nc, in_maps, core_ids=list(range(NH)), trace=_trace)
    except Exception:
        # transient NRT device errors recover on retry
        res = run_bass_kernel_spmd(
            nc, in_maps, core_ids=list(range(NH)), trace=_trace)
    LAST_EXEC_TIME_NS = res.exec_time_ns

    acc = np.zeros((N, D), dtype=np.float64)
    for h in range(NH):
        acc += np.asarray(res.results[h]["out"], dtype=np.float64)
    out = (acc + bproj.astype(np.float64)).astype(np.float32)
    return out.reshape(1, N, D)


# revision 15
# speedup vs baseline: 1.1165x; 1.1165x over previous
"""Multi-head attention (dense_transformer) on 8 TRN2 NeuronCores.

Reference computation (B=1, N=4096, D=512, 8 heads, head_dim 64):
    q = x @ Wq.T ; k, v = split(x @ Wkv.T)
    attn = softmax_masked(q k^T * scale)   # diagonal masked to zero
    out = (attn @ v) @ Wproj.T + bproj
Sharding: head-parallel - core h computes head h end to end including its
partial output projection; the host sums the 8 partials and adds the bias.

Per-core layout is fully "transposed" (channels on partitions):
    XT  [c=512, n=4096]    via 20 xbar DMA transposes
    QT/KT [128, 4096]      rows 0..63 = head projection, rows 64..127 = copy
                           (duplicated halves enable 2-way row-packed QK)
    ST group g = scores for key strips 2g, 2g+1 -> [128 keys, 2, 512 q] PSUM
    E = exp(scale * ST): split between ScalarE (exact table exp) and DVE
        (Schraudolph bitcast exp: bf16bits = round(A*s + B), one fused
        multiply-add at full DVE rate) so the two engines share the
        16.7M-element exp stream.  GpSimd zeroes the diagonal blocks.
    PV split-K: keys 0..63 and 64..127 of each strip feed two CONCURRENT
        row-tiled matmuls (PE quadrants (0,0) and (64,0)) accumulating
        OTa/OTb [65, 512] in separate PSUM banks; row 64 = sum of exps.
    merge: ScalarE evacuates OTb (+OTa's sums row), DVE adds PSUM+SBUF,
        GpSimd merges the sums rows.
    PO[tok, 512] = OTS[0:64, tok-block].T @ WprojSlice^T
    out[tok, :]  = PO * (1/sums)[tok]  (alternating DVE / ScalarE)
"""

import numpy as np

import concourse.bass as bass
import concourse.tile as tile
from concourse import bacc, mybir
from concourse.bass_utils import run_bass_kernel_spmd

F32 = mybir.dt.float32
BF16 = mybir.dt.bfloat16
I16 = mybir.dt.int16
EXP = mybir.ActivationFunctionType.Exp
COPY = mybir.ActivationFunctionType.Copy
MULT = mybir.AluOpType.mult
ADD = mybir.AluOpType.add

N = 4096
D = 512
NH = 8
HD = 64
NQC = 8          # query chunks of 512
QC = 512
NST = 32         # key strips of 128
GS = 2           # key strips per exp group
NG = NST // GS   # 16 groups per chunk

# Schraudolph bf16-bitcast exp: bf16_bits(exp(x)) ~ round(A16*x + B16)
A16 = 128.0 / float(np.log(2.0))
B16 = 16248.76   # mean-centered (HW-calibrated; RNE convert)

LAST_EXEC_TIME_NS = None
_BUILD_CACHE = {}


def _dve_exp_group(c, g):
    """Which exp groups run on the DVE (Schraudolph) vs ScalarE."""
    if c == 0:
        # chunk 0: DVE is busy evacuating the qt/kt/vp preamble
        return g % 4 == 3
    return g % 2 == 1


def _build(scale_val: float):
    nc = bacc.Bacc("TRN2", target_bir_lowering=False, debug=False)

    x_d = nc.dram_tensor("xinp", [N, D], BF16, kind="ExternalInput").ap()
    wq_d = nc.dram_tensor("wq", [128, 4, 128], BF16, kind="ExternalInput").ap()
    wk_d = nc.dram_tensor("wk", [128, 4, 128], BF16, kind="ExternalInput").ap()
    wv_d = nc.dram_tensor("wv", [128, 4, HD + 2], BF16, kind="ExternalInput").ap()
    mask_d = nc.dram_tensor("mask", [128, 128], BF16, kind="ExternalInput").ap()
    wp_d = nc.dram_tensor("wp", [HD, D], BF16, kind="ExternalInput").ap()
    identb_d = nc.dram_tensor("identb", [128, 128], BF16, kind="ExternalInput").ap()
    out_d = nc.dram_tensor("out", [N, D], BF16, kind="ExternalOutput").ap()

    a_s = A16 * scale_val

    with tile.TileContext(nc) as tc:
        with (
            tc.tile_pool(name="consts", bufs=1) as consts,
            tc.tile_pool(name="persist", bufs=1) as persist,
            tc.tile_pool(name="xin", bufs=8) as xin,
            tc.tile_pool(name="epool", bufs=6) as epool,
            tc.tile_pool(name="small", bufs=2) as small,
            tc.tile_pool(name="otbs", bufs=2) as otbs,
            tc.tile_pool(name="outp", bufs=4) as outp,
            tc.tile_pool(name="ps_st", bufs=2, space="PSUM") as ps_st,
            tc.tile_pool(name="ps_ot", bufs=1, space="PSUM") as ps_ot,
            tc.tile_pool(name="ps_misc", bufs=2, space="PSUM") as ps_misc,
        ):
            # ---- constants ----
            wq_sb = consts.tile([128, 4, 128], BF16, tag="wq")
            wk_sb = consts.tile([128, 4, 128], BF16, tag="wk")
            wv_sb = consts.tile([128, 4, HD + 2], BF16, tag="wv")
            mask_sb = consts.tile([128, 128], BF16, tag="mask")
            wp_sb = consts.tile([HD, D], BF16, tag="wp")
            identb_sb = consts.tile([128, 128], BF16, tag="identb")
            nc.sync.dma_start(out=wk_sb, in_=wk_d)
            nc.sync.dma_start(out=wq_sb, in_=wq_d)

            # ---- persistent tensors ----
            xt = persist.tile([128, 4, N], BF16, tag="xt")     # XT[c%128, c//128, n]
            qt = persist.tile([128, N], BF16, tag="qt")        # QT duplicated halves
            kt = persist.tile([128, N], BF16, tag="kt")
            vp = persist.tile([128, NST, HD + 2], BF16, tag="vp")

            # tokens 0..1023 go through the TensorEngine (their DMAs are
            # issued first so the xbar bulk transfer doesn't delay them);
            # tokens 1024..4095 arrive via xbar DMA transpose behind them.
            x_pre = []
            for t in range(8):
                x_t = xin.tile([128, D], BF16, tag="xin", name=f"x{t}")
                nc.sync.dma_start(out=x_t, in_=x_d[t * 128:(t + 1) * 128, :])
                x_pre.append(x_t)
            for b0 in range(1024, N, 1024):
                for cb in range(4):
                    nc.sync.dma_start_transpose(
                        out=xt[:, cb, b0:b0 + 1024],
                        in_=x_d[b0:b0 + 1024, cb * 128:(cb + 1) * 128],
                    )

            nc.sync.dma_start(out=wv_sb, in_=wv_d)
            nc.sync.dma_start(out=mask_sb, in_=mask_d)
            nc.sync.dma_start(out=wp_sb, in_=wp_d)
            nc.sync.dma_start(out=identb_sb, in_=identb_d)

            # preload the exp activation table while DMAs stream
            scratch = consts.tile([1, 2], F32, tag="scratch")
            nc.vector.memset(scratch, 0.0)
            nc.scalar.activation(scratch, scratch, EXP)
            # ones column of V' (row 64 of every strip) written once
            nc.vector.memset(vp[:, :, HD:HD + 1], 1.0)

            kt_done = [False] * NQC
            qt_done = [False] * NQC
            vq_done = [False] * (NST // 4)
            t_done = [False] * 8

            def prod_t(t):
                if t >= 8 or t_done[t]:
                    return
                t_done[t] = True
                x_t = x_pre[t]
                tr = ps_st.tile([128, D], BF16, tag="st", name=f"tr{t}")
                for cb in range(4):
                    nc.tensor.transpose(
                        tr[:, cb * 128:(cb + 1) * 128],
                        x_t[:, cb * 128:(cb + 1) * 128],
                        identb_sb,
                    )
                nc.vector.tensor_copy(
                    xt[:, 0:4, t * 128:(t + 1) * 128],
                    tr.rearrange("p (cb tt) -> p cb tt", cb=4),
                )

            def prod_kt(qc):
                if kt_done[qc]:
                    return
                kt_done[qc] = True
                for t in range(4 * qc, 4 * qc + 4):
                    prod_t(t)
                sl = slice(qc * QC, (qc + 1) * QC)
                pp = ps_misc.tile([128, QC], F32, tag="misc", name=f"ktp{qc}")
                for cc in range(4):
                    nc.tensor.matmul(
                        pp, wk_sb[:, cc, :], xt[:, cc, sl],
                        start=(cc == 0), stop=(cc == 3),
                    )
                nc.vector.tensor_copy(kt[:, sl], pp)

            def prod_qt(qc):
                if qt_done[qc]:
                    return
                qt_done[qc] = True
                for t in range(4 * qc, 4 * qc + 4):
                    prod_t(t)
                sl = slice(qc * QC, (qc + 1) * QC)
                pp = ps_misc.tile([128, QC], F32, tag="misc", name=f"qtp{qc}")
                for cc in range(4):
                    nc.tensor.matmul(
                        pp, wq_sb[:, cc, :], xt[:, cc, sl],
                        start=(cc == 0), stop=(cc == 3),
                    )
                nc.vector.tensor_copy(qt[:, sl], pp)

            def prod_vq(q):
                """V' for strips 4q..4q+3 in one PSUM tile + one copy."""
                if vq_done[q]:
                    return
                vq_done[q] = True
                for t in range(4 * q, 4 * q + 4):
                    prod_t(t)
                vv = ps_misc.tile([128, 4, HD + 2], F32, tag="misc",
                                  name=f"vv{q}")
                for t in range(4):
                    for cc in range(4):
                        nc.tensor.matmul(
                            vv[:, t, :],
                            xt[:, cc, (4 * q + t) * 128:(4 * q + t + 1) * 128],
                            wv_sb[:, cc, :],
                            start=(cc == 0), stop=(cc == 3),
                        )
                nc.vector.tensor_copy(
                    vp[:, 4 * q:4 * q + 4, 0:HD], vv[:, :, 0:HD])

            def prod_for_group(c, g):
                if c > 0 or g >= NG:
                    return
                prod_kt(g // 2)
                prod_vq(g // 2)

            # ---- per-chunk state ----
            ota_tiles = {}
            otb_tiles = {}
            e_tiles = {}
            st_tiles = {}
            sums_tiles = {}
            ots_tiles = {}

            def emit_qk(c, g):
                qsl = slice(c * QC, (c + 1) * QC)
                st = ps_st.tile([128, GS, QC], F32, tag="st")
                st_tiles[(c, g)] = st
                for i in range(GS):
                    j = GS * g + i
                    ro = 64 * (j % 2)
                    nc.tensor.matmul(
                        st[:, i, :],
                        kt[ro:ro + 64, j * 128:(j + 1) * 128],
                        qt[ro:ro + 64, qsl],
                        start=True,
                        stop=True,
                    )

            def emit_exp(c, g):
                e_t = epool.tile([128, GS, QC], BF16, tag="e")
                e_tiles[(c, g)] = e_t
                st = st_tiles.pop((c, g))
                if _dve_exp_group(c, g):
                    nc.vector.tensor_scalar(
                        e_t.bitcast(I16), st, a_s, B16, MULT, ADD)
                else:
                    nc.scalar.activation(e_t, st, EXP, scale=scale_val)
                for i in range(GS):
                    j = GS * g + i
                    r = j - 4 * c
                    if 0 <= r < 4:
                        blk = e_t[:, i, r * 128:(r + 1) * 128]
                        nc.gpsimd.tensor_tensor(blk, blk, mask_sb, MULT)

            def emit_pv(c, g):
                if g == 0:
                    ota_tiles[c] = ps_ot.tile([HD + 1, QC], F32, tag="ota",
                                              name=f"ota{c}")
                    otb_tiles[c] = ps_ot.tile([HD + 1, QC], F32, tag="otb",
                                              name=f"otb{c}")
                ota = ota_tiles[c]
                otb = otb_tiles[c]
                e_t = e_tiles.pop((c, g))
                for i in range(GS):
                    j = GS * g + i
                    nc.tensor.matmul(
                        ota,
                        vp[0:64, j, 0:HD + 1],
                        e_t[0:64, i, :],
                        start=(j == 0),
                        stop=(j == NST - 1),
                        skip_group_check=True,
                    )
                    nc.tensor.matmul(
                        otb,
                        vp[64:128, j, 0:HD + 1],
                        e_t[64:128, i, :],
                        start=(j == 0),
                        stop=(j == NST - 1),
                        skip_group_check=True,
                    )

            def emit_copies(c):
                # drain OTa/OTb out of PSUM; ScalarE evacuates the b half,
                # one DVE add merges rows 0..64 (row 64 = sums, bf16).
                ota = ota_tiles.pop(c)
                otb = otb_tiles.pop(c)
                otb_sb = otbs.tile([HD + 1, QC], F32, tag="otbs")
                nc.scalar.activation(otb_sb, otb, COPY)
                ots_sb = small.tile([HD, QC], BF16, tag="ots")
                nc.vector.tensor_tensor(
                    ots_sb, ota[0:HD, :], otb_sb[0:HD, :], ADD)
                sums_t = small.tile([HD + 1, QC], BF16, tag="sums")
                nc.vector.tensor_tensor(
                    sums_t[HD:HD + 1, :], ota[HD:HD + 1, :],
                    otb_sb[HD:HD + 1, :], ADD)
                sums_tiles[c] = sums_t
                ots_tiles[c] = ots_sb

            def emit_norm_po(c):
                pool = ps_st if c == NQC - 1 else ps_misc
                ptag = "st" if c == NQC - 1 else "misc"
                ots_sb = ots_tiles.pop(c)
                sums_t = sums_tiles.pop(c)
                ts_ps = pool.tile([128, 4, 2], BF16, tag=ptag, name=f"ts{c}")
                for tb in range(4):
                    nc.tensor.transpose(
                        ts_ps[:, tb, 0:1],
                        sums_t[HD:HD + 1, tb * 128:(tb + 1) * 128],
                        identb_sb[HD:HD + 1, HD:HD + 1],
                    )
                recip_sb = small.tile([128, 4], F32, tag="recip")
                with nc.allow_low_precision(reason="fp32 reciprocal"):
                    nc.vector.reciprocal(recip_sb, ts_ps[:, :, 0])
                for tb in range(4):
                    po = pool.tile([128, QC], F32, tag=ptag, name=f"po{c}_{tb}")
                    nc.tensor.matmul(
                        po,
                        ots_sb[:, tb * 128:(tb + 1) * 128],
                        wp_sb,
                        start=True,
                        stop=True,
                    )
                    o_sb = outp.tile([128, D], BF16, tag="outs")
                    if tb % 2 == 0:
                        nc.vector.tensor_scalar_mul(
                            o_sb, po, recip_sb[:, tb:tb + 1])
                    else:
                        nc.scalar.activation(
                            o_sb, po, COPY, scale=recip_sb[:, tb:tb + 1])
                    row = c * QC + tb * 128
                    nc.sync.dma_start(out=out_d[row:row + 128, :], in_=o_sb)

            # ---- flat software pipeline across all (chunk, group) steps ----
            seq = [(c, g) for c in range(NQC) for g in range(NG)]
            prod_qt(0)
            prod_for_group(0, 0)
            for i, (c, g) in enumerate(seq):
                emit_qk(c, g)
                if i > 1:
                    pc, pg = seq[i - 2]
                    emit_pv(pc, pg)
                    if pg == NG - 1:
                        emit_copies(pc)
                emit_exp(c, g)
                prod_for_group(c, g + 1)
                prod_for_group(c, g + 2)
                if i > 3:
                    ppc, ppg = seq[i - 4]
                    if ppg == NG - 1:
                        emit_norm_po(ppc)
                if g == NG // 2:
                    prod_qt(min(c + 1, NQC - 1))
            for i in (len(seq) - 2, len(seq) - 1):
                emit_pv(*seq[i])
            emit_copies(NQC - 1)
            emit_norm_po(NQC - 1)

    nc.compile()
    return nc


def _prep_inputs(x, scale, Wq, Wkv, Wproj):
    """Per-core input maps (head h on core h)."""
    import ml_dtypes
    bf = ml_dtypes.bfloat16
    x2 = np.ascontiguousarray(x.reshape(N, D)).astype(bf)
    mask = (1.0 - np.eye(128)).astype(np.float32)
    identb = np.eye(128, dtype=np.float32)
    in_maps = []
    for h in range(NH):
        wqh = Wq[h * HD:(h + 1) * HD, :]                  # [64, 512]
        wkh = Wkv[h * HD:(h + 1) * HD, :]
        wvh = Wkv[D + h * HD:D + (h + 1) * HD, :]
        # lhsT [c, m] with m duplicated halves -> [128, 4x128]
        def lhsT_dup(w):
            a = np.concatenate([w.T, w.T], axis=1)        # [512, 128]
            return np.ascontiguousarray(
                a.reshape(4, 128, 128).transpose(1, 0, 2))
        # V' rhs [c, 66] -> [128, 4, 66] (col 64 becomes the ones column)
        b = np.concatenate(
            [wvh.T, np.zeros((D, 2), dtype=np.float32)], axis=1)
        wv_host = np.ascontiguousarray(
            b.reshape(4, 128, HD + 2).transpose(1, 0, 2))
        wp_host = np.ascontiguousarray(
            Wproj[:, h * HD:(h + 1) * HD].T, dtype=np.float32)  # [64, 512]
        in_maps.append({
            "xinp": x2,
            "wq": np.ascontiguousarray(lhsT_dup(wqh)).astype(bf),
            "wk": np.ascontiguousarray(lhsT_dup(wkh)).astype(bf),
            "wv": np.ascontiguousarray(wv_host).astype(bf),
            "mask": mask.astype(bf),
            "wp": wp_host.astype(bf),
            "identb": identb.astype(bf),
        })
    return in_maps


def kernel(x, H, W, scale, Wq, Wkv, Wproj, bproj, _trace=False):
    global LAST_EXEC_TIME_NS
    x = np.asarray(x, dtype=np.float32)
    Wq = np.asarray(Wq, dtype=np.float32)
    Wkv = np.asarray(Wkv, dtype=np.float32)
    Wproj = np.asarray(Wproj, dtype=np.float32)
    bproj = np.asarray(bproj, dtype=np.float32)
    scale_val = float(np.asarray(scale).reshape(-1)[0])

    key = round(scale_val, 12)
    nc = _BUILD_CACHE.get(key)
    if nc is None:
        nc = _build(scale_val)
        _BUILD_CACHE[key] = nc

    in_maps = _prep_inputs(x, scale, Wq, Wkv, Wproj)
    try:
        res = run_bass_kernel_spmd(
            nc, in_maps, core_ids=list(range(NH)), trace=_trace)
    except Exception:
        # transient NRT device errors recover on retry
        res = run_bass_kernel_spmd(
            nc, in_maps, core_ids=list(range(NH)), trace=_trace)
    LAST_EXEC_TIME_NS = res.exec_time_ns

    acc = np.zeros((N, D), dtype=np.float64)
    for h in range(NH):
        acc += np.asarray(res.results[h]["out"], dtype=np.float64)
    out = (acc + bproj.astype(np.float64)).astype(np.float32)
    return out.reshape(1, N, D)
